# revision 7
# baseline (speedup 1.0000x reference)
"""Self-contained Trainium2 Bass kernel: DeBERTa-style disentangled MHA.

Model (per reference):
    q = x @ Wq.T + bq ; k = x @ Wk.T + bk ; v = x @ Wv.T + bv   (per-head split)
    pos_k = rel_emb @ Wk.T + bk ; pos_q = rel_emb @ Wq.T + bq
    scores[i,j] = (q_i.k_j + A[i, i-j+s] + B[j, i-j+s]) * scale + mask
        where A[i,t] = q_i . pos_k[t],  B[j,t] = k_j . pos_q[t]
    out = softmax_j(scores) @ v

Sharding: 8-way head-parallel (2 heads/core), every core handles all 8 batch rows.
Scores are computed transposed (k index on partitions) so probs feed the PV matmul
directly; the softmax denominator comes from an appended ones-column on V.
The relative-position diagonal gathers ("shear") go through a DRAM round trip:
windows are written with row pitch 640 and read back with row pitch 639, which
turns the per-row shift into a plain strided DMA.
"""

import numpy as np

B, S, DIM, H, HD = 8, 512, 1024, 16, 64
NCORES = 8
HPC = H // NCORES            # heads per core = 2
SCALE = float((HD * 3) ** -0.5)
W = 640                      # shear window width per 128-row tile
SEG = W * 128                # flat DRAM segment per tile

_prog_cache = {}


def _build_program():
    import concourse.bass as bass
    import concourse.mybir as mybir
    import concourse.tile as tile
    from concourse import bacc
    from concourse.masks import make_identity

    BF = mybir.dt.bfloat16
    F32 = mybir.dt.float32
    AO = mybir.AluOpType
    AF = mybir.ActivationFunctionType

    nc = bacc.Bacc(None, target_bir_lowering=False, debug=False)

    def ap_of(t, extra_off, dims):
        return bass.AP(t.tensor, int(t.offset) + extra_off, dims)

    names = {}

    with tile.TileContext(nc) as tc:
        with tc.tile_pool(name="dram", bufs=1, space="DRAM") as dram, \
             tc.tile_pool(name="const", bufs=1) as const, \
             tc.tile_pool(name="persist", bufs=1) as persist:

            # ---------------- I/O ----------------
            xT_d = dram.tile([DIM, B * S], BF, kind="ExternalInput", name="xT")
            relT_d = dram.tile([DIM, 2 * S], BF, kind="ExternalInput", name="relT")
            wqT_d = dram.tile([DIM, 128], BF, kind="ExternalInput", name="wqT")
            wkT_d = dram.tile([DIM, 128], BF, kind="ExternalInput", name="wkT")
            wvT_d = dram.tile([DIM, 128], BF, kind="ExternalInput", name="wvT")
            bq_d = dram.tile([128], F32, kind="ExternalInput", name="bq")
            bk_d = dram.tile([128], F32, kind="ExternalInput", name="bk")
            bv_d = dram.tile([128], F32, kind="ExternalInput", name="bv")
            mask_d = dram.tile([B, S], F32, kind="ExternalInput", name="mask")
            out_d = dram.tile([B * HPC, HD + 1, S], F32, kind="ExternalOutput",
                              name="out")
            for k, t in [("xT", xT_d), ("relT", relT_d), ("wqT", wqT_d),
                         ("wkT", wkT_d), ("wvT", wvT_d), ("bq", bq_d),
                         ("bk", bk_d), ("bv", bv_d), ("mask", mask_d),
                         ("out", out_d)]:
                names[k] = t.name

            # ---------------- persistent SBUF ----------------
            ident = const.tile([128, 128], BF)
            make_identity(nc, ident)
            bq_sb = const.tile([128, 1], F32)
            bk_sb = const.tile([128, 1], F32)
            bv_sb = const.tile([128, 1], F32)
            nc.sync.dma_start(out=bq_sb, in_=bq_d.rearrange("(p o) -> p o", o=1))
            nc.sync.dma_start(out=bk_sb, in_=bk_d.rearrange("(p o) -> p o", o=1))
            nc.sync.dma_start(out=bv_sb, in_=bv_d.rearrange("(p o) -> p o", o=1))
            # mask_sb[p, b*4+J] = mask[b, 128J + p]
            mask_sb = const.tile([128, B, 4], F32)
            nc.sync.dma_start(
                out=mask_sb,
                in_=ap_of(mask_d, 0, [[1, 128], [S, B], [128, 4]]))

            QT = persist.tile([128, B * S], BF)       # (x@WqT + bq)*scale, transposed
            KT = persist.tile([128, B * S], BF)       # x@WkT + bk, transposed
            posKTr = persist.tile([128, 2 * S], BF)   # pos_k^T, t-axis reversed
            posQT = persist.tile([128, 2 * S], BF)    # (pos_q^T)*scale
            # Vaug[:, b*4+J, 65h : 65h+65] = [v rows | ones] for PV lhsT
            Vaug = persist.tile([128, B * 4, 130], BF)
            nc.vector.memset(Vaug[:, :, 64:65], 1.0)
            nc.vector.memset(Vaug[:, :, 129:130], 1.0)

            # ---------------- setup phase ----------------
            with tc.tile_pool(name="wpool", bufs=1) as wpool, \
                 tc.tile_pool(name="xpool", bufs=1) as xpool, \
                 tc.tile_pool(name="setup_sb", bufs=1) as ssb, \
                 tc.tile_pool(name="setup_ps", bufs=2, space="PSUM") as sps:

                wq_sb = wpool.tile([128, 8, 128], BF)
                wk_sb = wpool.tile([128, 8, 128], BF)
                wv_sb = wpool.tile([128, 8, 128], BF)
                for wsb, wd in [(wq_sb, wqT_d), (wk_sb, wkT_d), (wv_sb, wvT_d)]:
                    nc.sync.dma_start(
                        out=wsb, in_=wd.rearrange("(k p) o -> p k o", p=128))

                xch = []
                for k in range(8):
                    t = xpool.tile([128, B * S], BF, name=f"xch{k}", tag=f"xch{k}")
                    nc.sync.dma_start(out=t, in_=xT_d[128 * k:128 * k + 128, :])
                    xch.append(t)
                relch = []
                for k in range(8):
                    t = ssb.tile([128, 2 * S], BF, name=f"relch{k}", tag=f"relch{k}")
                    nc.sync.dma_start(out=t, in_=relT_d[128 * k:128 * k + 128, :])
                    relch.append(t)

                VT_sb = ssb.tile([128, B * S], BF)

                # Q/K/V projections, transposed outputs [o=128, s]
                for st in range(8):
                    sl = slice(512 * st, 512 * st + 512)
                    psq = sps.tile([128, 512], F32, tag="psq")
                    psk = sps.tile([128, 512], F32, tag="psk")
                    psv = sps.tile([128, 512], F32, tag="psv")
                    for k in range(8):
                        fl = dict(start=(k == 0), stop=(k == 7))
                        nc.tensor.matmul(psq, wq_sb[:, k, :], xch[k][:, sl], **fl)
                        nc.tensor.matmul(psk, wk_sb[:, k, :], xch[k][:, sl], **fl)
                        nc.tensor.matmul(psv, wv_sb[:, k, :], xch[k][:, sl], **fl)
                    nc.vector.tensor_scalar(QT[:, sl], psq, bq_sb, SCALE, AO.add, AO.mult)
                    nc.vector.tensor_scalar_add(KT[:, sl], psk, bk_sb)
                    nc.vector.tensor_scalar_add(VT_sb[:, sl], psv, bv_sb)

                # pos projections [o=128, t=1024]
                posKT_tmp = ssb.tile([128, 2 * S], BF)
                for tt in range(2):
                    sl = slice(512 * tt, 512 * tt + 512)
                    pspk = sps.tile([128, 512], F32, tag="psq")
                    pspq = sps.tile([128, 512], F32, tag="psk")
                    for k in range(8):
                        fl = dict(start=(k == 0), stop=(k == 7))
                        nc.tensor.matmul(pspk, wk_sb[:, k, :], relch[k][:, sl], **fl)
                        nc.tensor.matmul(pspq, wq_sb[:, k, :], relch[k][:, sl], **fl)
                    nc.vector.tensor_scalar_add(posKT_tmp[:, sl], pspk, bk_sb)
                    nc.vector.tensor_scalar(posQT[:, sl], pspq, bq_sb, SCALE,
                                            AO.add, AO.mult)
                # reversed copy: posKTr[:, t] = posKT_tmp[:, 1023 - t]
                nc.vector.tensor_copy(
                    posKTr,
                    ap_of(posKT_tmp, 2 * S - 1, [[2 * S, 128], [-1, 2 * S]]))

                # V transposes -> Vaug
                for b in range(B):
                    for J in range(4):
                        pvt = sps.tile([128, 128], F32, tag="psv")
                        c0 = 512 * b + 128 * J
                        nc.tensor.matmul(pvt, VT_sb[:, c0:c0 + 128], ident,
                                         start=True, stop=True)
                        nc.vector.tensor_copy(Vaug[:, 4 * b + J, 0:64], pvt[:, 0:64])
                        nc.vector.tensor_copy(Vaug[:, 4 * b + J, 65:129], pvt[:, 64:128])

            # ---------------- attention phase ----------------
            with tc.tile_pool(name="work", bufs=1) as work, \
                 tc.tile_pool(name="dscratch", bufs=1, space="DRAM") as dscratch, \
                 tc.tile_pool(name="psab", bufs=2, space="PSUM") as psab, \
                 tc.tile_pool(name="psqk", bufs=2, space="PSUM") as psqk, \
                 tc.tile_pool(name="pspv", bufs=2, space="PSUM") as pspv:

                for b in range(B):
                    Asb, Bsb, aflat, bflat, c2p_sb, p2cT_sb = {}, {}, {}, {}, {}, {}
                    for h in range(HPC):
                        Asb[h] = work.tile([128, 4, W], BF, name=f"Asb{h}",
                                           tag=f"Asb{h}", bufs=2)
                        Bsb[h] = work.tile([128, 4, W], BF, name=f"Bsb{h}",
                                           tag=f"Bsb{h}", bufs=2)
                        aflat[h] = dscratch.tile([4 * SEG], BF, name=f"aflat{h}",
                                                 tag=f"aflat{h}", bufs=2)
                        bflat[h] = dscratch.tile([4 * SEG], BF, name=f"bflat{h}",
                                                 tag=f"bflat{h}", bufs=2)
                        c2p_sb[h] = work.tile([128, 4, 512], BF, name=f"c2p{h}",
                                              tag=f"c2p{h}", bufs=2)
                        p2cT_sb[h] = work.tile([128, 4, 512], BF, name=f"p2cT{h}",
                                               tag=f"p2cT{h}", bufs=2)

                    # A = q . pos_k_rev windows ; B = k . pos_q windows
                    for I in range(4):
                        w0 = 384 - 128 * I
                        for h in range(HPC):
                            hp = slice(64 * h, 64 * h + 64)
                            ps = psab.tile([128, W], F32, tag="psAB", bufs=2)
                            lq = QT[hp, 512 * b + 128 * I: 512 * b + 128 * I + 128]
                            nc.tensor.matmul(ps[:, 0:512], lq,
                                             posKTr[hp, w0:w0 + 512],
                                             start=True, stop=True)
                            nc.tensor.matmul(ps[:, 512:W], lq,
                                             posKTr[hp, w0 + 512:w0 + W],
                                             start=True, stop=True)
                            if h == 0:
                                nc.vector.tensor_copy(Asb[h][:, I, :], ps)
                            else:
                                nc.scalar.copy(Asb[h][:, I, :], ps)
                    for J in range(4):
                        w0 = 384 - 128 * J
                        for h in range(HPC):
                            hp = slice(64 * h, 64 * h + 64)
                            ps = psab.tile([128, W], F32, tag="psAB", bufs=2)
                            lk = KT[hp, 512 * b + 128 * J: 512 * b + 128 * J + 128]
                            nc.tensor.matmul(ps[:, 0:512], lk,
                                             posQT[hp, w0:w0 + 512],
                                             start=True, stop=True)
                            nc.tensor.matmul(ps[:, 512:W], lk,
                                             posQT[hp, w0 + 512:w0 + W],
                                             start=True, stop=True)
                            if h == 0:
                                nc.vector.tensor_copy(Bsb[h][:, J, :], ps)
                            else:
                                nc.scalar.copy(Bsb[h][:, J, :], ps)

                    # shear round trip
                    for h in range(HPC):
                        nc.sync.dma_start(
                            out=ap_of(aflat[h], 0, [[W, 128], [SEG, 4], [1, W]]),
                            in_=Asb[h][:])
                        nc.sync.dma_start(
                            out=c2p_sb[h][:],
                            in_=ap_of(aflat[h], 127, [[W - 1, 128], [SEG, 4], [1, 512]]))
                        nc.sync.dma_start(
                            out=ap_of(bflat[h], 0, [[W, 128], [SEG, 4], [1, W]]),
                            in_=Bsb[h][:])
                        nc.sync.dma_start(
                            out=p2cT_sb[h][:],
                            in_=ap_of(bflat[h], 128, [[W - 1, 128], [SEG, 4], [1, 512]]))

                    # scores (transposed), softmax, PV
                    for h in range(HPC):
                        hp = slice(64 * h, 64 * h + 64)
                        pvps = pspv.tile([65, 512], F32, tag="pv", bufs=2)
                        for J in range(4):
                            qkps = psqk.tile([128, 512], F32, tag="qk", bufs=2)
                            nc.tensor.matmul(
                                qkps,
                                KT[hp, 512 * b + 128 * J: 512 * b + 128 * J + 128],
                                QT[hp, 512 * b: 512 * b + 512],
                                start=True, stop=False)
                            for I in range(4):
                                nc.tensor.matmul(
                                    qkps[:, 128 * I:128 * I + 128],
                                    c2p_sb[h][:, I, 128 * J:128 * J + 128],
                                    ident, start=False, stop=False,
                                    skip_group_check=True)
                            nc.tensor.matmul(qkps, ident, p2cT_sb[h][:, J, :],
                                             start=False, stop=True)
                            PT = work.tile([128, 512], BF, tag="PT", bufs=3)
                            nc.scalar.activation(
                                PT, qkps, AF.Exp,
                                bias=mask_sb[:, b, J:J + 1], scale=1.0)
                            nc.tensor.matmul(pvps, Vaug[:, 4 * b + J, 65 * h:65 * h + 65],
                                             PT, start=(J == 0), stop=(J == 3))
                        outsb = work.tile([65, 512], F32, tag="outsb", bufs=2)
                        nc.vector.tensor_copy(outsb, pvps)
                        nc.sync.dma_start(out=out_d[HPC * b + h], in_=outsb)

    nc.compile()
    return nc, names


def _get_program():
    if "prog" not in _prog_cache:
        _prog_cache["prog"] = _build_program()
    return _prog_cache["prog"]


def _host_prep(x, rel_embeddings, attn_mask, Wq, bq, Wk, bk, Wv, bv):
    import ml_dtypes
    bf = ml_dtypes.bfloat16
    x = np.asarray(x, np.float32)
    xT = np.ascontiguousarray(x.reshape(B * S, DIM).T).astype(bf)
    relT = np.ascontiguousarray(np.asarray(rel_embeddings, np.float32).T).astype(bf)
    WqT = np.asarray(Wq, np.float32).T
    WkT = np.asarray(Wk, np.float32).T
    WvT = np.asarray(Wv, np.float32).T
    mask = np.ascontiguousarray(
        np.asarray(attn_mask, np.float32).reshape(B, S))
    bq = np.asarray(bq, np.float32)
    bk = np.asarray(bk, np.float32)
    bv = np.asarray(bv, np.float32)
    maps = []
    for c in range(NCORES):
        sl = slice(128 * c, 128 * c + 128)
        maps.append({
            "xT": xT,
            "relT": relT,
            "wqT": np.ascontiguousarray(WqT[:, sl]).astype(bf),
            "wkT": np.ascontiguousarray(WkT[:, sl]).astype(bf),
            "wvT": np.ascontiguousarray(WvT[:, sl]).astype(bf),
            "bq": np.ascontiguousarray(bq[sl]),
            "bk": np.ascontiguousarray(bk[sl]),
            "bv": np.ascontiguousarray(bv[sl]),
            "mask": mask,
        })
    return maps


def kernel(x, rel_embeddings, attn_mask, Wq, bq, Wk, bk, Wv, bv):
    from concourse.bass_utils import run_bass_kernel_spmd

    nc, names = _get_program()
    maps = _host_prep(x, rel_embeddings, attn_mask, Wq, bq, Wk, bk, Wv, bv)
    in_maps = [{names[k]: v for k, v in m.items()} for m in maps]
    res = run_bass_kernel_spmd(nc, in_maps, list(range(NCORES)))
    out = np.empty((B, S, DIM), np.float32)
    for c in range(NCORES):
        o = np.asarray(res.results[c][names["out"]], np.float32)
        for b in range(B):
            for hl in range(HPC):
                d0 = 128 * c + 64 * hl
                blk = o[HPC * b + hl]          # [65, 512]: rows 0-63 PV, row 64 L
                out[b, :, d0:d0 + 64] = (blk[0:64] / blk[64:65]).T
    return out


# revision 12
# speedup vs baseline: 1.0417x; 1.0417x over previous
"""Self-contained Trainium2 Bass kernel: DeBERTa-style disentangled MHA.

Model (per reference):
    q = x @ Wq.T + bq ; k = x @ Wk.T + bk ; v = x @ Wv.T + bv   (per-head split)
    pos_k = rel_emb @ Wk.T + bk ; pos_q = rel_emb @ Wq.T + bq
    scores[i,j] = (q_i.k_j + A[i, i-j+s] + B[j, i-j+s]) * scale + mask
        where A[i,t] = q_i . pos_k[t],  B[j,t] = k_j . pos_q[t]
    out = softmax_j(scores) @ v

Sharding: 8-way head-parallel (2 heads/core), every core handles all 8 batch rows.
Scores are computed transposed (k index on partitions) so probs feed the PV matmul
directly; the softmax denominator comes from an appended ones-column on V.
The relative-position diagonal gathers ("shear") go through a DRAM round trip:
windows are written with row pitch 640 and read back with row pitch 639, which
turns the per-row shift into a plain strided DMA.
"""

import numpy as np

B, S, DIM, H, HD = 8, 512, 1024, 16, 64
NCORES = 8
HPC = H // NCORES            # heads per core = 2
SCALE = float((HD * 3) ** -0.5)
W = 640                      # shear window width per 128-row tile
SEG = W * 128                # flat DRAM segment per tile

_prog_cache = {}


def _build_program():
    import concourse.bass as bass
    import concourse.mybir as mybir
    import concourse.tile as tile
    from concourse import bacc
    from concourse.masks import make_identity

    BF = mybir.dt.bfloat16
    F32 = mybir.dt.float32
    AO = mybir.AluOpType
    AF = mybir.ActivationFunctionType

    nc = bacc.Bacc(None, target_bir_lowering=False, debug=False)

    def ap_of(t, extra_off, dims):
        return bass.AP(t.tensor, int(t.offset) + extra_off, dims)

    names = {}

    with tile.TileContext(nc) as tc:
        with tc.tile_pool(name="dram", bufs=1, space="DRAM") as dram, \
             tc.tile_pool(name="const", bufs=1) as const, \
             tc.tile_pool(name="persist", bufs=1) as persist:

            # ---------------- I/O ----------------
            xT_d = dram.tile([DIM, B * S], BF, kind="ExternalInput", name="xT")
            relT_d = dram.tile([DIM, 2 * S], BF, kind="ExternalInput", name="relT")
            wqT_d = dram.tile([DIM, 128], BF, kind="ExternalInput", name="wqT")
            wkT_d = dram.tile([DIM, 128], BF, kind="ExternalInput", name="wkT")
            wvT_d = dram.tile([DIM, 128], BF, kind="ExternalInput", name="wvT")
            bq_d = dram.tile([128], F32, kind="ExternalInput", name="bq")
            bk_d = dram.tile([128], F32, kind="ExternalInput", name="bk")
            bv_d = dram.tile([128], F32, kind="ExternalInput", name="bv")
            mask_d = dram.tile([B, S], F32, kind="ExternalInput", name="mask")
            out_d = dram.tile([B * HPC, HD + 1, S], F32, kind="ExternalOutput",
                              name="out")
            for k, t in [("xT", xT_d), ("relT", relT_d), ("wqT", wqT_d),
                         ("wkT", wkT_d), ("wvT", wvT_d), ("bq", bq_d),
                         ("bk", bk_d), ("bv", bv_d), ("mask", mask_d),
                         ("out", out_d)]:
                names[k] = t.name

            # ---------------- persistent SBUF ----------------
            ident = const.tile([128, 128], BF)
            make_identity(nc, ident)
            bq_sb = const.tile([128, 1], F32)
            bk_sb = const.tile([128, 1], F32)
            bv_sb = const.tile([128, 1], F32)
            nc.sync.dma_start(out=bq_sb, in_=bq_d.rearrange("(p o) -> p o", o=1))
            nc.sync.dma_start(out=bk_sb, in_=bk_d.rearrange("(p o) -> p o", o=1))
            nc.sync.dma_start(out=bv_sb, in_=bv_d.rearrange("(p o) -> p o", o=1))
            # mask_sb[p, b*4+J] = mask[b, 128J + p]
            mask_sb = const.tile([128, B, 4], F32)
            nc.sync.dma_start(
                out=mask_sb,
                in_=ap_of(mask_d, 0, [[1, 128], [S, B], [128, 4]]))

            QT = persist.tile([128, B * S], BF)       # (x@WqT + bq)*scale, transposed
            KT = persist.tile([128, B * S], BF)       # x@WkT + bk, transposed
            posKTr = persist.tile([128, 2 * S], BF)   # pos_k^T, t-axis reversed
            posQT = persist.tile([128, 2 * S], BF)    # (pos_q^T)*scale
            # Vaug[:, b*4+J, 65h : 65h+65] = [v rows | ones] for PV lhsT
            Vaug = persist.tile([128, B * 4, 130], BF)
            nc.vector.memset(Vaug[:, :, 64:65], 1.0)
            nc.vector.memset(Vaug[:, :, 129:130], 1.0)

            # ---------------- setup phase ----------------
            with tc.tile_pool(name="wpool", bufs=1) as wpool, \
                 tc.tile_pool(name="xpool", bufs=1) as xpool, \
                 tc.tile_pool(name="setup_sb", bufs=1) as ssb, \
                 tc.tile_pool(name="setup_ps", bufs=2, space="PSUM") as sps:

                wq_sb = wpool.tile([128, 8, 128], BF)
                wk_sb = wpool.tile([128, 8, 128], BF)
                wv_sb = wpool.tile([128, 8, 128], BF)
                for wsb, wd in [(wq_sb, wqT_d), (wk_sb, wkT_d), (wv_sb, wvT_d)]:
                    nc.sync.dma_start(
                        out=wsb, in_=wd.rearrange("(k p) o -> p k o", p=128))

                xch = []
                for k in range(8):
                    t = xpool.tile([128, B * S], BF, name=f"xch{k}", tag=f"xch{k}")
                    nc.sync.dma_start(out=t, in_=xT_d[128 * k:128 * k + 128, :])
                    xch.append(t)
                relch = []
                for k in range(8):
                    t = ssb.tile([128, 2 * S], BF, name=f"relch{k}", tag=f"relch{k}")
                    nc.sync.dma_start(out=t, in_=relT_d[128 * k:128 * k + 128, :])
                    relch.append(t)

                VT_sb = ssb.tile([128, B * S], BF)

                # Q/K/V projections, transposed outputs [o=128, s]
                for st in range(8):
                    sl = slice(512 * st, 512 * st + 512)
                    psq = sps.tile([128, 512], F32, tag="psq")
                    psk = sps.tile([128, 512], F32, tag="psk")
                    psv = sps.tile([128, 512], F32, tag="psv")
                    for k in range(8):
                        fl = dict(start=(k == 0), stop=(k == 7))
                        nc.tensor.matmul(psq, wq_sb[:, k, :], xch[k][:, sl], **fl)
                        nc.tensor.matmul(psk, wk_sb[:, k, :], xch[k][:, sl], **fl)
                        nc.tensor.matmul(psv, wv_sb[:, k, :], xch[k][:, sl], **fl)
                    nc.vector.tensor_scalar(QT[:, sl], psq, bq_sb, SCALE, AO.add, AO.mult)
                    nc.vector.tensor_scalar_add(KT[:, sl], psk, bk_sb)
                    nc.vector.tensor_scalar_add(VT_sb[:, sl], psv, bv_sb)

                # pos projections [o=128, t=1024]
                posKT_tmp = ssb.tile([128, 2 * S], BF)
                for tt in range(2):
                    sl = slice(512 * tt, 512 * tt + 512)
                    pspk = sps.tile([128, 512], F32, tag="psq")
                    pspq = sps.tile([128, 512], F32, tag="psk")
                    for k in range(8):
                        fl = dict(start=(k == 0), stop=(k == 7))
                        nc.tensor.matmul(pspk, wk_sb[:, k, :], relch[k][:, sl], **fl)
                        nc.tensor.matmul(pspq, wq_sb[:, k, :], relch[k][:, sl], **fl)
                    nc.vector.tensor_scalar_add(posKT_tmp[:, sl], pspk, bk_sb)
                    nc.vector.tensor_scalar(posQT[:, sl], pspq, bq_sb, SCALE,
                                            AO.add, AO.mult)
                # reversed copy: posKTr[:, t] = posKT_tmp[:, 1023 - t]
                nc.vector.tensor_copy(
                    posKTr,
                    ap_of(posKT_tmp, 2 * S - 1, [[2 * S, 128], [-1, 2 * S]]))

                # V transposes -> Vaug
                for b in range(B):
                    for J in range(4):
                        pvt = sps.tile([128, 128], F32, tag="psv")
                        c0 = 512 * b + 128 * J
                        nc.tensor.matmul(pvt, VT_sb[:, c0:c0 + 128], ident,
                                         start=True, stop=True)
                        nc.vector.tensor_copy(Vaug[:, 4 * b + J, 0:64], pvt[:, 0:64])
                        nc.vector.tensor_copy(Vaug[:, 4 * b + J, 65:129], pvt[:, 64:128])

            # ---------------- attention phase ----------------
            with tc.tile_pool(name="work", bufs=1) as work, \
                 tc.tile_pool(name="dscratch", bufs=1, space="DRAM") as dscratch, \
                 tc.tile_pool(name="psab", bufs=2, space="PSUM") as psab, \
                 tc.tile_pool(name="psqk", bufs=2, space="PSUM") as psqk, \
                 tc.tile_pool(name="pspv", bufs=2, space="PSUM") as pspv:

                for b in range(B):
                    # ABsb[h][:, 0] holds A windows, [:, 1] holds B windows
                    ABsb, abflat, gath = {}, {}, {}
                    for h in range(HPC):
                        ABsb[h] = work.tile([128, 2, 4, W], BF, name=f"ABsb{h}",
                                            tag=f"ABsb{h}", bufs=3)
                        abflat[h] = dscratch.tile([2 * 4 * SEG], BF,
                                                  name=f"abflat{h}",
                                                  tag=f"abflat{h}", bufs=3)
                        # gath[h][:, 0] = c2p (natural), [:, 1] = p2c^T
                        gath[h] = work.tile([128, 2, 4, 512], BF, name=f"gath{h}",
                                            tag=f"gath{h}", bufs=3)

                    # A = q . pos_k_rev windows ; B = k . pos_q windows
                    # (two heads issued back-to-back: K=64 row-group packing)
                    for I in range(4):
                        w0 = 384 - 128 * I
                        ps = {}
                        for h in range(HPC):
                            hp = slice(64 * h, 64 * h + 64)
                            ps[h] = psab.tile([128, W], F32, name=f"psAB{h}", tag=f"psAB{h}", bufs=1)
                            lq = QT[hp, 512 * b + 128 * I: 512 * b + 128 * I + 128]
                            nc.tensor.matmul(ps[h][:, 0:512], lq,
                                             posKTr[hp, w0:w0 + 512],
                                             start=True, stop=True)
                            nc.tensor.matmul(ps[h][:, 512:W], lq,
                                             posKTr[hp, w0 + 512:w0 + W],
                                             start=True, stop=True)
                        nc.vector.tensor_copy(ABsb[0][:, 0, I, :], ps[0])
                        nc.scalar.copy(ABsb[1][:, 0, I, :], ps[1])
                    for J in range(4):
                        w0 = 384 - 128 * J
                        ps = {}
                        for h in range(HPC):
                            hp = slice(64 * h, 64 * h + 64)
                            ps[h] = psab.tile([128, W], F32, name=f"psAB{h}", tag=f"psAB{h}", bufs=1)
                            lk = KT[hp, 512 * b + 128 * J: 512 * b + 128 * J + 128]
                            nc.tensor.matmul(ps[h][:, 0:512], lk,
                                             posQT[hp, w0:w0 + 512],
                                             start=True, stop=True)
                            nc.tensor.matmul(ps[h][:, 512:W], lk,
                                             posQT[hp, w0 + 512:w0 + W],
                                             start=True, stop=True)
                        nc.vector.tensor_copy(ABsb[0][:, 1, J, :], ps[0])
                        nc.scalar.copy(ABsb[1][:, 1, J, :], ps[1])

                    # shear round trip: one contiguous write + one strided
                    # gather-read per head.  Writes go through SWDGE (gpsimd)
                    # to keep the SP sequencer free for the reads.
                    for h in range(HPC):
                        nc.gpsimd.dma_start(
                            out=ap_of(abflat[h], 0,
                                      [[W, 128], [4 * SEG, 2], [SEG, 4], [1, W]]),
                            in_=ABsb[h][:])
                        nc.sync.dma_start(
                            out=gath[h][:, 0],
                            in_=ap_of(abflat[h], 127,
                                      [[W - 1, 128], [SEG, 4], [1, 512]]))
                        nc.sync.dma_start(
                            out=gath[h][:, 1],
                            in_=ap_of(abflat[h], 4 * SEG + 128,
                                      [[W - 1, 128], [SEG, 4], [1, 512]]))

                    # scores (transposed), softmax, PV
                    for h in range(HPC):
                        hp = slice(64 * h, 64 * h + 64)
                        pvps = pspv.tile([65, 512], F32, tag="pv", bufs=2)
                        for J in range(4):
                            qkps = psqk.tile([128, 512], F32, tag="qk", bufs=2)
                            nc.tensor.matmul(
                                qkps,
                                KT[hp, 512 * b + 128 * J: 512 * b + 128 * J + 128],
                                QT[hp, 512 * b: 512 * b + 512],
                                start=True, stop=False)
                            for I in range(4):
                                nc.tensor.matmul(
                                    qkps[:, 128 * I:128 * I + 128],
                                    gath[h][:, 0, I, 128 * J:128 * J + 128],
                                    ident, start=False, stop=False,
                                    skip_group_check=True)
                            nc.tensor.matmul(qkps, ident, gath[h][:, 1, J, :],
                                             start=False, stop=True)
                            PT = work.tile([128, 512], BF, tag="PT", bufs=4)
                            nc.scalar.activation(
                                PT, qkps, AF.Exp,
                                bias=mask_sb[:, b, J:J + 1], scale=1.0)
                            nc.tensor.matmul(pvps, Vaug[:, 4 * b + J, 65 * h:65 * h + 65],
                                             PT, start=(J == 0), stop=(J == 3))
                        outsb = work.tile([65, 512], F32, tag="outsb", bufs=2)
                        nc.vector.tensor_copy(outsb, pvps)
                        nc.scalar.dma_start(out=out_d[HPC * b + h], in_=outsb)

    nc.compile()
    return nc, names


def _get_program():
    if "prog" not in _prog_cache:
        _prog_cache["prog"] = _build_program()
    return _prog_cache["prog"]


def _host_prep(x, rel_embeddings, attn_mask, Wq, bq, Wk, bk, Wv, bv):
    import ml_dtypes
    bf = ml_dtypes.bfloat16
    x = np.asarray(x, np.float32)
    xT = np.ascontiguousarray(x.reshape(B * S, DIM).T).astype(bf)
    relT = np.ascontiguousarray(np.asarray(rel_embeddings, np.float32).T).astype(bf)
    WqT = np.asarray(Wq, np.float32).T
    WkT = np.asarray(Wk, np.float32).T
    WvT = np.asarray(Wv, np.float32).T
    mask = np.ascontiguousarray(
        np.asarray(attn_mask, np.float32).reshape(B, S))
    bq = np.asarray(bq, np.float32)
    bk = np.asarray(bk, np.float32)
    bv = np.asarray(bv, np.float32)
    maps = []
    for c in range(NCORES):
        sl = slice(128 * c, 128 * c + 128)
        maps.append({
            "xT": xT,
            "relT": relT,
            "wqT": np.ascontiguousarray(WqT[:, sl]).astype(bf),
            "wkT": np.ascontiguousarray(WkT[:, sl]).astype(bf),
            "wvT": np.ascontiguousarray(WvT[:, sl]).astype(bf),
            "bq": np.ascontiguousarray(bq[sl]),
            "bk": np.ascontiguousarray(bk[sl]),
            "bv": np.ascontiguousarray(bv[sl]),
            "mask": mask,
        })
    return maps


def kernel(x, rel_embeddings, attn_mask, Wq, bq, Wk, bk, Wv, bv):
    from concourse.bass_utils import run_bass_kernel_spmd

    nc, names = _get_program()
    maps = _host_prep(x, rel_embeddings, attn_mask, Wq, bq, Wk, bk, Wv, bv)
    in_maps = [{names[k]: v for k, v in m.items()} for m in maps]
    res = run_bass_kernel_spmd(nc, in_maps, list(range(NCORES)))
    out = np.empty((B, S, DIM), np.float32)
    for c in range(NCORES):
        o = np.asarray(res.results[c][names["out"]], np.float32)
        for b in range(B):
            for hl in range(HPC):
                d0 = 128 * c + 64 * hl
                blk = o[HPC * b + hl]          # [65, 512]: rows 0-63 PV, row 64 L
                out[b, :, d0:d0 + 64] = (blk[0:64] / blk[64:65]).T
    return out
